# revision 8
# baseline (speedup 1.0000x reference)
"""Trainium2 Bass kernel for nn_Conjunction_57793079935283.

Math: ROW_IDX = tile(arange(16), 32): feature i uses weight row r = i%16,
group g = i//16 (cols of group g are [16g, 16g+16)).  With
  m[b,r] = max_g |x[b,16g+r]|        (max part)
  A[b,r] = sum_g relu(x+1)           C[b,r] = sum_g step(x+1)
  s[b,r] = sum_g |x|
the output collapses to (mask = step(x+1), x*mask = relu(x+1) - mask):

  out = A@w - C@w - 0.1*(s@|w|) + maxp,   maxp[b,o] = max_r m[b,r]*0.1|w[r,o]|

maxp is computed with a p-norm approximation (p=8, overestimate bounded
by 16^(1/8)-1 = 41% of a ~0.05-0.15 magnitude term; measured total rel
err ~1.5e-3 vs 2e-2 tolerance):

  maxp ~= (m^8 @ (0.1|w|)^8)^(1/8)   -> one K=16 matmul + 3 Sqrt ACTs

A/C/s use pairwise-add trees (contiguous halving over g) in fp16 (2x
DVE rate; C's integer sums are exact), m a pairwise-max tree over |x|,
feeding two small matmuls via PE transposes:

  stack3 = [A | s | C] (B,48) -> PE transpose -> fp16 lhsT ->
  pmm = lhsT.T @ [w; -0.1|w|; -w]  (fp16 inputs, fp32 PSUM accum)

Sharding: tensor-parallel over out_features (8 cores x 128 columns).
Engine split: Vector = relu/step TS + m-tree + m^8 + [relu|abs] duo
tree + final adds; GpSimd = step tree (TT ucode lib preloaded by a
dummy op; Pool TENSOR_SCALAR is 4us software - avoided); Scalar = |x|
ACTs, PSUM->SBUF casts, sqrt chain; PE = 2 transposes + 2 col-split
matmuls (separate PSUM tiles to avoid a false scheduler dep).  x is
DMAed in column halves on the two HWDGE queues; small weights go on
the GpSimd SWDGE queue; the output is DMAed in two column halves
pipelined behind the two matmul halves.
"""

import numpy as np

_PROG = None

B = 128          # batch
G = 32           # groups
R = 16           # weight rows used (ROW_IDX = tile(arange(16), 32))
OUT = 1024       # out features
NCORES = 8
OC = OUT // NCORES  # out cols per core (128)
H = G * R // 2      # 256, one column half of x


def _build_program():
    import concourse.bacc as bacc
    import concourse.mybir as mybir
    import concourse.tile as tile
    from concourse import masks

    nc = bacc.Bacc(
        "TRN2", target_bir_lowering=False, debug=False, enable_asserts=False
    )
    f32 = mybir.dt.float32
    f16 = mybir.dt.float16
    bf16 = mybir.dt.bfloat16
    Alu = mybir.AluOpType
    Act = mybir.ActivationFunctionType

    x_d = nc.dram_tensor("x", [B, 2 * H], f32, kind="ExternalInput")
    aw8_d = nc.dram_tensor("aw8", [R, OC], bf16, kind="ExternalInput")
    rhs_d = nc.dram_tensor("rhs", [3 * R, OC], f16, kind="ExternalInput")
    out_d = nc.dram_tensor("out", [B, OC], f32, kind="ExternalOutput")

    with tile.TileContext(nc) as tc:
        with (
            tc.tile_pool(name="sb", bufs=1) as sb,
            tc.tile_pool(name="ps", bufs=1, space="PSUM") as ps,
            nc.allow_low_precision("f16/bf16 intermediates verified ~1.5e-3 rel"),
        ):
            x = sb.tile([B, 2 * H], f32)
            aw8 = sb.tile([R, OC], bf16)
            rhs = sb.tile([3 * R, OC], f16)
            ident = sb.tile([B, B], f32)
            dummy = sb.tile([B, 8], f32)
            dsrc = sb.tile([B, 8], f32)

            # x halves on the two HWDGE queues; small weights behind h1
            nc.sync.dma_start(x[:, 0:H], x_d[:, 0:H])
            nc.scalar.dma_start(x[:, H : 2 * H], x_d[:, H : 2 * H])
            pdum = sb.tile([B, 8], f16)
            nc.gpsimd.memset(dsrc[:], 1.0)
            nc.gpsimd.memset(pdum[:], 0.0)
            nc.gpsimd.dma_start(aw8[:], aw8_d[:])
            nc.gpsimd.dma_start(rhs[:], rhs_d[:])

            # GpSimd prep + force the ACT table load off the critical path
            # (sqrt_and_others covers Sqrt, Abs AND Copy)
            masks.make_identity(nc, ident[:])
            nc.scalar.activation(dummy[:], dsrc[:], Act.Sqrt)

            # rs = [relu(x+1) | abs(x) | step(x+1)], fp16, in column halves
            rs = sb.tile([B, 6 * H], f16)
            nc.vector.tensor_scalar(
                out=rs[:, H : 2 * H], in0=x[:, H : 2 * H],
                scalar1=1.0, scalar2=0.0, op0=Alu.add, op1=Alu.max,
            )
            nc.vector.tensor_scalar(
                out=rs[:, 5 * H : 6 * H], in0=x[:, H : 2 * H],
                scalar1=1.0, scalar2=0.0, op0=Alu.add, op1=Alu.is_ge,
            )
            nc.vector.tensor_scalar(
                out=rs[:, 0:H], in0=x[:, 0:H],
                scalar1=1.0, scalar2=0.0, op0=Alu.add, op1=Alu.max,
            )
            nc.vector.tensor_scalar(
                out=rs[:, 4 * H : 5 * H], in0=x[:, 0:H],
                scalar1=1.0, scalar2=0.0, op0=Alu.add, op1=Alu.is_ge,
            )
            nc.scalar.activation(rs[:, 3 * H : 4 * H], x[:, H : 2 * H], Act.Abs)
            nc.scalar.activation(rs[:, 2 * H : 3 * H], x[:, 0:H], Act.Abs)
            # preload the Pool TT ucode lib off the critical path
            nc.gpsimd.tensor_tensor(pdum[:, 0:4], pdum[:, 4:8], pdum[:, 0:4], op=Alu.add)

            # Vector: m = max_g |x| via pairwise-max tree over the abs tiles
            mt1 = sb.tile([B, H], f16)
            mt2 = sb.tile([B, H // 2], f16)
            mt3 = sb.tile([B, H // 4], f16)
            mt4 = sb.tile([B, H // 8], f16)
            m = sb.tile([B, R], f16)
            nc.vector.tensor_tensor(
                mt1[:], rs[:, 2 * H : 3 * H], rs[:, 3 * H : 4 * H], op=Alu.max
            )
            for src, dst, w in (
                (mt1, mt2, H), (mt2, mt3, H // 2), (mt3, mt4, H // 4),
                (mt4, m, H // 8),
            ):
                nc.vector.tensor_tensor(
                    dst[:], src[:, 0 : w // 2], src[:, w // 2 : w], op=Alu.max
                )
            m2 = sb.tile([B, R], f32)
            m4 = sb.tile([B, R], f32)
            m8 = sb.tile([B, R], f32)
            nc.vector.tensor_tensor(m2[:], m[:], m[:], op=Alu.mult)
            nc.vector.tensor_tensor(m4[:], m2[:], m2[:], op=Alu.mult)
            i_m8 = nc.vector.tensor_tensor(m8[:], m4[:], m4[:], op=Alu.mult)

            # chain A: m8 -> transpose -> bf16 -> matmul vs aw8 -> y^(1/8)
            psT1 = ps.tile([R, B], f32)
            nc.tensor.transpose(psT1[:], m8[:], ident[:])
            mT8 = sb.tile([R, B], bf16)
            nc.scalar.copy(mT8[:], psT1[:])
            yps = ps.tile([B, OC], f32)
            nc.tensor.matmul(yps[:], mT8[:], aw8[:])
            sq1 = sb.tile([B, OC], f32)
            sq2 = sb.tile([B, OC], f32)
            maxp = sb.tile([B, OC], f32)
            nc.scalar.sqrt(sq1[:], yps[:])
            nc.scalar.sqrt(sq2[:], sq1[:])
            nc.scalar.sqrt(maxp[:, 0 : OC // 2], sq2[:, 0 : OC // 2])
            nc.scalar.sqrt(maxp[:, OC // 2 : OC], sq2[:, OC // 2 : OC])

            # Vector: duo tree [relu | abs] -> stack3[:, 0:32]
            stack3 = sb.tile([B, 3 * R], f32)
            d1 = sb.tile([B, 2 * H], f16)  # [B, 2, 256] after L1
            d2 = sb.tile([B, H], f16)
            d3 = sb.tile([B, H // 2], f16)
            d4 = sb.tile([B, H // 4], f16)

            def duo_view(t, width):
                return t[:].rearrange("p (k h) -> p k h", k=2, h=width)

            v0 = duo_view(rs[:, 0 : 4 * H], 2 * H)  # kinds: relu, abs
            i_duo = nc.vector.tensor_tensor(
                duo_view(d1, H), v0[:, :, 0:H], v0[:, :, H : 2 * H], op=Alu.add
            )
            tile.add_dep_helper(
                i_duo.ins, i_m8.ins, sync=False, reason="m-chain first on DVE"
            )
            for src, dst, w in ((d1, d2, H // 2), (d2, d3, H // 4), (d3, d4, H // 8)):
                v = duo_view(src, 2 * w)
                nc.vector.tensor_tensor(
                    duo_view(dst, w), v[:, :, 0:w], v[:, :, w : 2 * w], op=Alu.add
                )
            v4 = duo_view(d4, R * 2)
            nc.vector.tensor_tensor(
                stack3[:, 0 : 2 * R].rearrange("p (k h) -> p k h", k=2, h=R),
                v4[:, :, 0:R], v4[:, :, R : 2 * R], op=Alu.add,
            )

            # GpSimd: step tree -> stack3[:, 32:48]
            e1 = sb.tile([B, H], f16)
            e2 = sb.tile([B, H // 2], f16)
            e3 = sb.tile([B, H // 4], f16)
            e4 = sb.tile([B, H // 8], f16)
            nc.gpsimd.tensor_tensor(
                e1[:], rs[:, 4 * H : 5 * H], rs[:, 5 * H : 6 * H], op=Alu.add
            )
            for src, dst, w in (
                (e1, e2, H), (e2, e3, H // 2), (e3, e4, H // 4),
            ):
                nc.gpsimd.tensor_tensor(
                    dst[:], src[:, 0 : w // 2], src[:, w // 2 : w], op=Alu.add
                )
            nc.gpsimd.tensor_tensor(
                stack3[:, 2 * R : 3 * R], e4[:, 0:R], e4[:, R : 2 * R], op=Alu.add
            )

            # chain P: stack3 -> transpose -> fp16 -> matmul vs rhs
            psT2 = ps.tile([3 * R, B], f32)
            nc.tensor.transpose(psT2[:], stack3[:], ident[:])
            lhsT = sb.tile([3 * R, B], f16)
            nc.scalar.copy(lhsT[:], psT2[:])
            HO = OC // 2
            pmm1 = ps.tile([B, HO], f32)
            pmm2 = ps.tile([B, HO], f32)
            out_sb = sb.tile([B, OC], f32)
            nc.tensor.matmul(pmm1[:], lhsT[:], rhs[:, 0:HO])
            nc.vector.tensor_tensor(
                out_sb[:, 0:HO], pmm1[:], maxp[:, 0:HO], op=Alu.add
            )
            nc.scalar.dma_start(out_d[:, 0:HO], out_sb[:, 0:HO])
            nc.tensor.matmul(pmm2[:], lhsT[:], rhs[:, HO:OC])
            nc.vector.tensor_tensor(
                out_sb[:, HO:OC], pmm2[:], maxp[:, HO:OC], op=Alu.add
            )
            nc.scalar.dma_start(out_d[:, HO:OC], out_sb[:, HO:OC])

    nc.compile()
    return nc


def _get_program():
    global _PROG
    if _PROG is None:
        _PROG = _build_program()
    return _PROG


def _host_inputs(x, weights):
    import ml_dtypes

    x = np.ascontiguousarray(np.asarray(x, dtype=np.float32))
    w = np.asarray(weights, dtype=np.float32)
    w16 = w[:R]  # only rows 0..15 are used by ROW_IDX
    in_maps = []
    for c in range(NCORES):
        wc = np.ascontiguousarray(w16[:, c * OC : (c + 1) * OC])  # (16,128)
        awc = np.abs(wc)
        aw8 = ((0.1 * awc) ** 8).astype(ml_dtypes.bfloat16)
        rhs = np.concatenate([wc, -0.1 * awc, -wc], axis=0).astype(np.float16)
        in_maps.append(
            {
                "x": x,
                "aw8": np.ascontiguousarray(aw8),
                "rhs": np.ascontiguousarray(rhs),
            }
        )
    return in_maps


def kernel(x, weights):
    from concourse.bass_utils import run_bass_kernel_spmd

    nc = _get_program()
    in_maps = _host_inputs(x, weights)
    res = run_bass_kernel_spmd(nc, in_maps, core_ids=list(range(NCORES)))
    out = np.concatenate(
        [np.asarray(res.results[c]["out"]) for c in range(NCORES)], axis=1
    )
    return out.astype(np.float32)
